# revision 1
# baseline (speedup 1.0000x reference)
"""Trainium2 Bass kernel for a dense pre-norm transformer block.

B, S, H, NH, MLP = 4, 2048, 768, 12, 3072 (fp32 I/O).

Sharding: 8 shards = (batch, seq-half). Each core receives its batch's full
2048-token sequence with its own 1024 query tokens permuted to the front
(attention is permutation-invariant over keys), computes K/V for all 2048
tokens, and Q/attention/MLP for its 1024 query tokens. No collectives.

On-chip: activations are kept feature-major [feature-part, token-free] for
matmuls (weights stationary), token-major for LN/softmax-normalize/residual.
Attention computes scoresT = K @ Q^T per head, exponentiates on ACT
(scale=1/8 folded), then multiplies with a stationary [V | ones] so the
softmax denominator accumulates for free in the extra PSUM row; the
normalization happens after a PE transpose back to token-major where the
denominator is a per-partition scalar. bf16 matmul inputs, fp32 accumulation,
fp32 LN/residual spine.

Schedule: the Q/K projections are interleaved with attention per head-pair so
the tensor engine never idles waiting on ACT exp (keeps the PE clock-gate
warm); PSUM->SBUF copies ride on DVE to keep ACT free for exp.
"""

import sys

if "/opt/trn_rl_repo" not in sys.path:
    sys.path.insert(0, "/opt/trn_rl_repo")

from contextlib import ExitStack

import ml_dtypes
import numpy as np

import concourse.bacc as bacc
import concourse.bass as bass
import concourse.mybir as mybir
import concourse.tile as tile
from concourse.alu_op_type import AluOpType
from concourse.bass_utils import run_bass_kernel_spmd
from concourse.masks import make_identity

B, S, H, NH, MLPD = 4, 2048, 768, 12, 3072
HD = H // NH  # 64
EPS = 1e-6
P = 128
N_H = H // P  # 6
N_M = MLPD // P  # 24
AF = mybir.ActivationFunctionType
BF = mybir.dt.bfloat16
F32 = mybir.dt.float32

_BUILD_CACHE = {}


def build(tkv=S, mlp_act="Gelu"):
    key = (tkv, mlp_act)
    if key in _BUILD_CACHE:
        return _BUILD_CACHE[key]

    tq = tkv // 2
    n_kv = tkv // P  # K/V token tiles
    n_q = tq // P  # query token tiles
    CH = 512 if tq % 512 == 0 else tq  # moving-operand chunk
    n_cq = tq // CH  # query chunks
    n_ckv = tkv // CH  # kv chunks
    n_b = CH // P  # 128-blocks per chunk
    VC = 384  # v-proj output chunk (6 heads)
    n_vc = H // VC  # 2

    nc = bacc.Bacc("TRN2", target_bir_lowering=False, debug=False, num_devices=8)

    x_d = nc.dram_tensor("x_loc", (tkv, H), F32, kind="ExternalInput").ap()
    wq_d = nc.dram_tensor("wq", (H, H), BF, kind="ExternalInput").ap()
    wk_d = nc.dram_tensor("wk", (H, H), BF, kind="ExternalInput").ap()
    wv_d = nc.dram_tensor("wv", (H, H), BF, kind="ExternalInput").ap()
    wo_d = nc.dram_tensor("wo", (H, H), BF, kind="ExternalInput").ap()
    w1_d = nc.dram_tensor("w1", (H, MLPD), BF, kind="ExternalInput").ap()
    w2_d = nc.dram_tensor("w2", (MLPD, H), BF, kind="ExternalInput").ap()
    bq_d = nc.dram_tensor("bq", (H,), F32, kind="ExternalInput").ap()
    bk_d = nc.dram_tensor("bk", (H,), F32, kind="ExternalInput").ap()
    bv_d = nc.dram_tensor("bv", (H,), BF, kind="ExternalInput").ap()
    bo_d = nc.dram_tensor("bo", (H,), F32, kind="ExternalInput").ap()
    b1_d = nc.dram_tensor("b1", (MLPD,), F32, kind="ExternalInput").ap()
    b2_d = nc.dram_tensor("b2", (H,), F32, kind="ExternalInput").ap()
    ln1w_d = nc.dram_tensor("ln1_w", (H,), BF, kind="ExternalInput").ap()
    ln1b_d = nc.dram_tensor("ln1_b", (H,), BF, kind="ExternalInput").ap()
    ln2w_d = nc.dram_tensor("ln2_w", (H,), BF, kind="ExternalInput").ap()
    ln2b_d = nc.dram_tensor("ln2_b", (H,), BF, kind="ExternalInput").ap()
    out_d = nc.dram_tensor("out_loc", (tq, H), F32, kind="ExternalOutput").ap()

    def bcast(ap1d):
        return bass.AP(
            tensor=ap1d.tensor, offset=ap1d.offset, ap=[[0, P]] + list(ap1d.ap)
        )

    with tile.TileContext(nc) as tc, ExitStack() as top:
        const = top.enter_context(tc.tile_pool(name="const", bufs=1))
        persist = top.enter_context(tc.tile_pool(name="persist", bufs=1))
        # Top-level PSUM pool: 2 banks shared by transposes + proj accums.
        psum = top.enter_context(tc.tile_pool(name="psum", bufs=1, space="PSUM"))
        toks = top.enter_context(tc.tile_pool(name="toks", bufs=4))
        tmps = top.enter_context(tc.tile_pool(name="tmps", bufs=2))

        # ---- constants ----
        ident = const.tile([P, P], BF)
        make_identity(nc, ident)
        eps_t = const.tile([P, 1], F32)
        nc.vector.memset(eps_t, EPS)
        ln1w_bc = const.tile([P, H], BF)
        nc.gpsimd.dma_start(out=ln1w_bc, in_=bcast(ln1w_d))
        ln1b_bc = const.tile([P, H], BF)
        nc.gpsimd.dma_start(out=ln1b_bc, in_=bcast(ln1b_d))
        ln2w_bc = const.tile([P, H], BF)
        nc.gpsimd.dma_start(out=ln2w_bc, in_=bcast(ln2w_d))
        ln2b_bc = const.tile([P, H], BF)
        nc.gpsimd.dma_start(out=ln2b_bc, in_=bcast(ln2b_d))
        bv_row = const.tile([1, H], BF)
        nc.sync.dma_start(out=bv_row, in_=bv_d[None, :])
        ones_row = const.tile([1, P], BF)
        nc.vector.memset(ones_row, 1.0)
        bq_sb = const.tile([P, N_H], F32)
        nc.sync.dma_start(out=bq_sb, in_=bq_d.rearrange("(t p) -> p t", p=P))
        bk_sb = const.tile([P, N_H], F32)
        nc.sync.dma_start(out=bk_sb, in_=bk_d.rearrange("(t p) -> p t", p=P))
        bo_sb = const.tile([P, N_H], F32)
        nc.sync.dma_start(out=bo_sb, in_=bo_d.rearrange("(t p) -> p t", p=P))
        b1_sb = const.tile([P, N_M], F32)
        nc.sync.dma_start(out=b1_sb, in_=b1_d.rearrange("(t p) -> p t", p=P))
        b2_sb = const.tile([P, N_H], F32)
        nc.sync.dma_start(out=b2_sb, in_=b2_d.rearrange("(t p) -> p t", p=P))
        wo_sb = const.tile([P, N_H, H], BF)

        ctx_tok = persist.tile([P, n_q, H], BF)  # normalized ctx (token-major)

        def ln_tile(x_ap, w_bc, b_bc, out_bf):
            """LayerNorm of one [P, H] fp32 tile -> bf16 out (token-major).

            Stats on DVE, the normalize pass on ACT (per-partition
            scale/bias), the weight/bias application on DVE in bf16.
            """
            stats = tmps.tile([P, 2, 6], F32, tag="ln_stats", bufs=4)
            for g in range(2):
                nc.vector.bn_stats(out=stats[:, g, :], in_=x_ap[:, g * 384 : (g + 1) * 384])
            mv = tmps.tile([P, 2], F32, tag="ln_mv", bufs=4)
            nc.vector.bn_aggr(out=mv, in_=stats)
            rstd = tmps.tile([P, 1], F32, tag="ln_rstd", bufs=4)
            nc.scalar.activation(out=rstd, in_=mv[:, 1:2], func=AF.Sqrt, bias=eps_t, scale=1.0)
            nc.vector.reciprocal(out=rstd, in_=rstd)
            nmr = tmps.tile([P, 1], F32, tag="ln_nmr", bufs=4)
            nc.vector.scalar_tensor_tensor(
                out=nmr, in0=mv[:, 0:1], scalar=-1.0, in1=rstd,
                op0=AluOpType.mult, op1=AluOpType.mult,
            )
            xh = tmps.tile([P, H], BF, tag="ln_xh", bufs=4)
            nc.scalar.activation(out=xh, in_=x_ap, func=AF.Identity, scale=rstd, bias=nmr)
            nc.vector.tensor_mul(out_bf, xh, w_bc)
            nc.vector.tensor_add(out_bf, out_bf, b_bc)

        def transpose_to(dst_ap, src_ap, rows, cols):
            """dst[cols, rows] = src[rows, cols].T (both SBUF bf16)."""
            pt = psum.tile([P, P], BF, tag="aux", bufs=2)
            nc.tensor.transpose(pt[0:cols, 0:rows], src_ap, ident[0:rows, 0:rows])
            nc.scalar.copy(out=dst_ap, in_=pt[0:cols, 0:rows])

        # ====== Phase 1-3: LN1, V proj, then per head-pair (QK proj +
        # attention) so PE-dense projection work fills exp-wait gaps. ======
        ACH = min(1024, tq)
        n_ac = tq // ACH
        n_sc = ACH // CH
        with tc.tile_pool(name="qkv_sb", bufs=1) as qkv_sb:
            # Q stored zero-padded per head: head h occupies its 64 rows,
            # the other 64 rows stay zero, so the scores matmul can use the
            # full 128-row kT stationary (FWL) with exact math.
            qT = qkv_sb.tile([P, NH, tq], BF)
            nc.vector.memset(qT, 0.0)
            kT = qkv_sb.tile([P, N_H, tkv], BF)
            vone = qkv_sb.tile([P, n_kv, NH, HD + 1], BF)
            nc.vector.memset(vone[:, :, :, HD : HD + 1], 1.0)

            with tc.tile_pool(name="ln_qkv", bufs=1) as lnp, tc.tile_pool(
                name="attn_sb", bufs=1
            ) as asb:
                xnT = lnp.tile([P, N_H, tkv], BF)
                wq_sb = lnp.tile([P, N_H, H], BF)
                wk_sb = lnp.tile([P, N_H, H], BF)
                wv_sb = lnp.tile([P, N_H, H], BF)
                for i in range(N_H):
                    nc.sync.dma_start(out=wv_sb[:, i, :], in_=wv_d[i * P : (i + 1) * P, :])

                # LN1 + transpose + V projection, per token tile (keeps PE
                # fed with V matmuls while DVE/ACT chew the next LN). The
                # attention PSUM pool opens only after this loop, so V
                # accumulators and transposes get their own banks here.
                with tc.tile_pool(name="psLN", bufs=1, space="PSUM") as psLN:
                    for t in range(n_kv):
                        x_t = toks.tile([P, H], F32, tag="xtok")
                        nc.sync.dma_start(out=x_t, in_=x_d[t * P : (t + 1) * P, :])
                        xn_bf = tmps.tile([P, H], BF, tag="xn_bf", bufs=4)
                        ln_tile(x_t, ln1w_bc, ln1b_bc, xn_bf)
                        for j in range(N_H):
                            transpose_to(
                                xnT[:, j, t * P : (t + 1) * P],
                                xn_bf[:, j * P : (j + 1) * P], P, P,
                            )
                        for c2 in range(n_vc):
                            pv = psLN.tile([P, VC], F32, tag="pv", bufs=3)
                            # bias row via K=1 ones-matmul, then accumulate
                            nc.tensor.matmul(
                                pv, ones_row[:, 0:P],
                                bv_row[:, c2 * VC : (c2 + 1) * VC],
                                start=True, stop=False,
                            )
                            for hit in range(N_H):
                                nc.tensor.matmul(
                                    pv,
                                    xnT[:, hit, t * P : (t + 1) * P],
                                    wv_sb[:, hit, c2 * VC : (c2 + 1) * VC],
                                    start=False, stop=(hit == N_H - 1),
                                )
                            nc.vector.tensor_copy(
                                out=vone[:, t, c2 * (VC // HD) : (c2 + 1) * (VC // HD), 0:HD],
                                in_=pv.rearrange("p (h d) -> p h d", d=HD),
                            )

                # Q/K/O weights only needed once attention starts; emit
                # their loads after the x/LN traffic so they don't delay it.
                for i in range(N_H):
                    nc.sync.dma_start(out=wq_sb[:, i, :], in_=wq_d[i * P : (i + 1) * P, :])
                    nc.sync.dma_start(out=wk_sb[:, i, :], in_=wk_d[i * P : (i + 1) * P, :])
                    nc.sync.dma_start(out=wo_sb[:, i, :], in_=wo_d[i * P : (i + 1) * P, :])

                def qk_proj(w_sb, b_sb, dstT, hot, n_c, split_q=False):
                    for c in range(n_c):
                        pk = psum.tile([P, CH], F32, tag="aux", bufs=2)
                        for hit in range(N_H):
                            nc.tensor.matmul(
                                pk,
                                w_sb[:, hit, hot * P : (hot + 1) * P],
                                xnT[:, hit, c * CH : (c + 1) * CH],
                                start=(hit == 0), stop=(hit == N_H - 1),
                            )
                        if split_q:
                            nc.vector.tensor_scalar_add(
                                dstT[0:HD, 2 * hot, c * CH : (c + 1) * CH],
                                pk[0:HD, :], b_sb[:, hot : hot + 1][0:HD],
                            )
                            nc.vector.tensor_scalar_add(
                                dstT[HD:P, 2 * hot + 1, c * CH : (c + 1) * CH],
                                pk[HD:P, :], b_sb[:, hot : hot + 1][HD:P],
                            )
                        else:
                            nc.vector.tensor_scalar_add(
                                dstT[:, hot, c * CH : (c + 1) * CH], pk,
                                b_sb[:, hot : hot + 1],
                            )

                psA = []

                def attention_head(h):
                    hr = (h % 2) * HD
                    ht = h // 2
                    for c in range(n_ac):
                        pctx = psA[0].tile([P, ACH], F32, tag="pctx", bufs=1)
                        for kt in range(n_kv):
                            ps = psA[0].tile([P, ACH], F32, tag="psc", bufs=2)
                            for sc in range(n_sc):
                                nc.tensor.matmul(
                                    ps[:, sc * CH : (sc + 1) * CH],
                                    kT[:, ht, kt * P : (kt + 1) * P],
                                    qT[:, h,
                                       c * ACH + sc * CH : c * ACH + (sc + 1) * CH],
                                    start=True, stop=True,
                                )
                            ex = asb.tile([P, ACH], BF, tag="exp", bufs=8)
                            nc.scalar.activation(out=ex, in_=ps, func=AF.Exp, scale=0.125)
                            for sc in range(n_sc):
                                nc.tensor.matmul(
                                    pctx[0 : HD + 1, sc * CH : (sc + 1) * CH],
                                    vone[:, kt, h, :],
                                    ex[:, sc * CH : (sc + 1) * CH],
                                    start=(kt == 0), stop=(kt == n_kv - 1),
                                )
                        cd = asb.tile([P, ACH], BF, tag="cd", bufs=3)
                        nc.vector.tensor_copy(out=cd[0 : HD + 1, :], in_=pctx[0 : HD + 1, :])
                        for b4 in range(ACH // P):
                            t_tok = c * (ACH // P) + b4
                            pt = psum.tile([P, P], BF, tag="aux", bufs=2)
                            nc.tensor.transpose(
                                pt[0:P, 0 : HD + 1],
                                cd[0 : HD + 1, b4 * P : (b4 + 1) * P],
                                ident[0 : HD + 1, 0 : HD + 1],
                            )
                            rp = tmps.tile([P, 1], F32, tag="rp", bufs=4)
                            nc.vector.reciprocal(rp, pt[:, HD : HD + 1])
                            nc.vector.tensor_scalar_mul(
                                ctx_tok[:, t_tok, h * HD : (h + 1) * HD],
                                pt[:, 0:HD],
                                rp,
                            )

                # interleave: QK-proj for pair ht, attention on pair ht, then
                # fold the pair's out-projection contribution into u_acc.
                with tc.tile_pool(name="psA", bufs=1, space="PSUM") as psA_:
                    psA.append(psA_)
                    for ht in range(N_H):
                        qk_proj(wq_sb, bq_sb, qT, ht, n_cq, split_q=True)
                        qk_proj(wk_sb, bk_sb, kT, ht, n_ckv)
                        attention_head(2 * ht)
                        attention_head(2 * ht + 1)

        # ========== Phase 4-6 ==========
        with tc.tile_pool(name="late", bufs=1) as late:
            x1_sb = late.tile([P, n_q, H], F32)  # attn-block out (token-major)

            # ---- ctx transpose, out-proj, residual (per tq-chunk) ----
            with tc.tile_pool(name="oproj", bufs=1) as op:
                ctxT = op.tile([P, N_H, tq], BF)
                uT = op.tile([P, N_H, tq], BF)
                for c in range(n_cq):
                    for t in range(c * n_b, (c + 1) * n_b):
                        for j in range(N_H):
                            transpose_to(
                                ctxT[:, j, t * P : (t + 1) * P],
                                ctx_tok[:, t, j * P : (j + 1) * P], P, P,
                            )
                    for hot in range(N_H):
                        pu = psum.tile([P, CH], F32, tag="aux", bufs=2)
                        for hit in range(N_H):
                            nc.tensor.matmul(
                                pu,
                                wo_sb[:, hit, hot * P : (hot + 1) * P],
                                ctxT[:, hit, c * CH : (c + 1) * CH],
                                start=(hit == 0), stop=(hit == N_H - 1),
                            )
                        nc.vector.tensor_scalar_add(
                            uT[:, hot, c * CH : (c + 1) * CH], pu,
                            bo_sb[:, hot : hot + 1],
                        )
                    for t in range(c * n_b, (c + 1) * n_b):
                        xr = toks.tile([P, H], F32, tag="xtok")
                        nc.sync.dma_start(out=xr, in_=x_d[t * P : (t + 1) * P, :])
                        for j in range(N_H):
                            pt = psum.tile([P, P], BF, tag="aux", bufs=2)
                            nc.tensor.transpose(
                                pt, uT[:, j, t * P : (t + 1) * P], ident,
                            )
                            nc.vector.tensor_add(
                                x1_sb[:, t, j * P : (j + 1) * P],
                                pt,
                                xr[:, j * P : (j + 1) * P],
                            )

            # ---- LN2 + MLP ----
            with tc.tile_pool(name="mlp_sb", bufs=1) as mp, tc.tile_pool(
                name="ps6", bufs=1, space="PSUM"
            ) as ps6:
                xn2T = mp.tile([P, N_H, tq], BF)
                w1_sb = mp.tile([P, N_H, MLPD], BF)
                w2_sb = mp.tile([P, N_M, H], BF)
                h1c = mp.tile([P, N_M, CH], BF)
                y2T = mp.tile([P, N_H, CH], BF)
                for i in range(N_H):
                    nc.sync.dma_start(out=w1_sb[:, i, :], in_=w1_d[i * P : (i + 1) * P, :])
                for i in range(N_M):
                    nc.sync.dma_start(out=w2_sb[:, i, :], in_=w2_d[i * P : (i + 1) * P, :])

                for t in range(n_q):
                    xn2_bf = tmps.tile([P, H], BF, tag="xn_bf", bufs=4)
                    ln_tile(x1_sb[:, t, :], ln2w_bc, ln2b_bc, xn2_bf)
                    for j in range(N_H):
                        transpose_to(
                            xn2T[:, j, t * P : (t + 1) * P],
                            xn2_bf[:, j * P : (j + 1) * P], P, P,
                        )

                for c in range(n_cq):
                    for mt in range(N_M):
                        ph = ps6.tile([P, CH], F32, tag="pmm", bufs=4)
                        for hit in range(N_H):
                            nc.tensor.matmul(
                                ph,
                                w1_sb[:, hit, mt * P : (mt + 1) * P],
                                xn2T[:, hit, c * CH : (c + 1) * CH],
                                start=(hit == 0), stop=(hit == N_H - 1),
                            )
                        nc.scalar.activation(
                            out=h1c[:, mt, :], in_=ph,
                            func=getattr(AF, mlp_act), bias=b1_sb[:, mt : mt + 1],
                        )
                    for hot in range(N_H):
                        py = ps6.tile([P, CH], F32, tag="pmm", bufs=4)
                        for mt in range(N_M):
                            nc.tensor.matmul(
                                py,
                                w2_sb[:, mt, hot * P : (hot + 1) * P],
                                h1c[:, mt, :],
                                start=(mt == 0), stop=(mt == N_M - 1),
                            )
                        nc.vector.tensor_scalar_add(
                            y2T[:, hot, :], py, b2_sb[:, hot : hot + 1],
                        )
                    for b4 in range(n_b):
                        t = c * n_b + b4
                        outt = toks.tile([P, H], F32, tag="xtok")
                        for j in range(N_H):
                            pt = psum.tile([P, P], BF, tag="aux", bufs=2)
                            nc.tensor.transpose(
                                pt, y2T[:, j, b4 * P : (b4 + 1) * P], ident,
                            )
                            nc.vector.tensor_add(
                                outt[:, j * P : (j + 1) * P],
                                pt,
                                x1_sb[:, t, j * P : (j + 1) * P],
                            )
                        nc.sync.dma_start(out=out_d[t * P : (t + 1) * P, :], in_=outt)

    nc.compile()
    _BUILD_CACHE[key] = nc
    return nc


def make_in_maps(inputs, tkv=S):
    """Build the 8 per-core input maps from full inputs."""
    f = np.asarray
    x = f(inputs["x"], dtype=np.float32)
    tq = tkv // 2
    wcast = {
        n: np.ascontiguousarray(f(inputs[n]).astype(ml_dtypes.bfloat16))
        for n in ["wq", "wk", "wv", "wo", "w1", "w2"]
    }
    fp32v = {
        n: np.ascontiguousarray(f(inputs[n], dtype=np.float32))
        for n in ["bq", "bk", "bo", "b1", "b2"]
    }
    for n in ["ln1_w", "ln1_b", "ln2_w", "ln2_b", "bv"]:
        fp32v[n] = np.ascontiguousarray(f(inputs[n]).astype(ml_dtypes.bfloat16))
    in_maps = []
    for c in range(8):
        b, half = c // 2, c % 2
        if half == 0:
            x_loc = x[b, :tkv]
        else:
            x_loc = np.concatenate([x[b, tq:tkv], x[b, :tq]], axis=0)
        m = {"x_loc": np.ascontiguousarray(x_loc)}
        m.update(wcast)
        m.update(fp32v)
        in_maps.append(m)
    return in_maps


def kernel(**inputs):
    nc = build(S)
    in_maps = make_in_maps(inputs, S)
    res = run_bass_kernel_spmd(nc, in_maps, core_ids=list(range(8)))
    tq = S // 2
    out = np.empty((B, S, H), dtype=np.float32)
    for c in range(8):
        b, half = c // 2, c % 2
        out[b, half * tq : (half + 1) * tq] = res.results[c]["out_loc"]
    return out

